# revision 21
# baseline (speedup 1.0000x reference)
"""Trainium2 Bass kernel for a 2-layer GRU decoder (B=128, T=512, H=512).

Sharding: data-parallel on batch across 8 cores (16 batch elems/core).
Recurrence layout: hidden state kept transposed
[128 partitions = hidden-dim within chunk, kappa(4) chunks, batch(16)];
U is the stationary matmul operand in bf16 (fast weight load), h the
moving operand, so gate math runs in the same layout the next step's
matmul consumes — no per-step transposes.
"""

import numpy as np
import ml_dtypes

B, T, LAT, F2, H = 128, 512, 256, 64, 512
DIN = LAT + F2          # 320
G3 = 3 * H              # 1536
NCORES = 8
BL = B // NCORES        # 16 batch / core
NT = T * BL             # 8192 flat (t,b) rows per core
NCH = G3 // 128         # 12 output-dim chunks
KH = H // 128           # 4 hidden-dim chunks
KIN = 3                 # padded 384 input-dim chunks
NBULK = NT // 512       # 16 bulk column chunks of 512
TB = T // NBULK         # 32 timesteps per bulk chunk
NJ = NT // 128          # 64 dense output chunks
UNROLL = 8

bf16 = ml_dtypes.bfloat16
_CACHE = {}


def _build(bd_val, has_brh):
    import concourse.bass as bass
    import concourse.tile as tile
    import concourse.mybir as mybir
    from concourse import bacc
    from concourse.bass import ds

    f32 = mybir.dt.float32
    bf = mybir.dt.bfloat16
    AF = mybir.ActivationFunctionType
    OP = mybir.AluOpType
    ET = mybir.EngineType

    nc = bacc.Bacc(None, target_bir_lowering=False, debug=False)

    xt_d = nc.dram_tensor("xt_d", [KIN, 128, NT], bf, kind="ExternalInput")
    w5_d = nc.dram_tensor("w5_d", [KIN, 128, G3], bf, kind="ExternalInput")
    u5_d = nc.dram_tensor("u5_d", [KH, 128, G3], bf, kind="ExternalInput")
    w6_d = nc.dram_tensor("w6_d", [KH, 128, G3], bf, kind="ExternalInput")
    u6_d = nc.dram_tensor("u6_d", [KH, 128, G3], bf, kind="ExternalInput")
    b5_d = nc.dram_tensor("b5_d", [128, NCH], f32, kind="ExternalInput")
    b6_d = nc.dram_tensor("b6_d", [128, NCH], f32, kind="ExternalInput")
    br5_d = nc.dram_tensor("br5_d", [128, KH], f32, kind="ExternalInput")
    br6_d = nc.dram_tensor("br6_d", [128, KH], f32, kind="ExternalInput")
    wd_d = nc.dram_tensor("wd_d", [128, KH], bf, kind="ExternalInput")
    dm_d = nc.dram_tensor("dm_d", [128, NJ], f32, kind="ExternalInput")
    out_d = nc.dram_tensor("out_d", [128, NJ], f32, kind="ExternalOutput")

    with tile.TileContext(nc) as tc:
        import contextlib
        stack = contextlib.ExitStack()
        with stack:
            drp = stack.enter_context(tc.tile_pool(name="dram", bufs=1, space="DRAM"))
            xw5_t = drp.tile([128, T, NCH, BL], bf)
            xw6_t = drp.tile([128, T, NCH, BL], bf)
            g5_t = drp.tile([128, KH, T, BL], bf)
            g6_t = drp.tile([128, KH, T, BL], bf)

            pp = stack.enter_context(tc.tile_pool(name="persist", bufs=1))
            h_f32 = pp.tile([128, KH * BL], f32, tag="hf")
            h_bf = pp.tile([128, KH, BL], bf, tag="hb")
            b5_sb = pp.tile([128, NCH], f32, tag="b5")
            b6_sb = pp.tile([128, NCH], f32, tag="b6")
            nc.sync.dma_start(b5_sb[:], b5_d.ap()[:])
            nc.sync.dma_start(b6_sb[:], b6_d.ap()[:])
            # pre-touch on DVE so later ops need no extra DMA sync waits
            scr = pp.tile([128, 1], f32, tag="scr")
            nc.vector.tensor_tensor(
                scr[:], b5_sb[:, 0:1], b6_sb[:, 0:1],
                op=mybir.AluOpType.add,
            )
            if has_brh:
                brh5_sb = pp.tile([128, KH], f32, tag="brh5")
                brh6_sb = pp.tile([128, KH], f32, tag="brh6")
                nc.sync.dma_start(brh5_sb[:], br5_d.ap()[:])
                nc.sync.dma_start(brh6_sb[:], br6_d.ap()[:])

            # ---------- Phase 1: xw5 = (X @ W5) * scale + bias ----------
            def bulk_phase(w_d, src_sb_getter, xw_t, bias_sb, kdim):
                with (
                    tc.tile_pool(name="blkw", bufs=1) as wp,
                    tc.tile_pool(name="blkps", bufs=4, space="PSUM") as psp,
                    tc.tile_pool(name="blko", bufs=2) as op_,
                ):
                    w_sb = wp.tile([128, kdim, G3], bf, tag="wsb")
                    nc.sync.dma_start(
                        w_sb[:], w_d.ap().rearrange("k p g -> p k g")
                    )
                    for n in range(NBULK):
                        ob = op_.tile([128, TB, NCH, BL], bf, tag="ob")
                        for c in range(NCH):
                            ps = psp.tile([128, 512], f32, tag="ps")
                            for k in range(kdim):
                                nc.tensor.matmul(
                                    ps[:],
                                    w_sb[:, k, c * 128:(c + 1) * 128],
                                    src_sb_getter(k, n),
                                    start=(k == 0),
                                    stop=(k == kdim - 1),
                                )
                            nc.vector.tensor_scalar(
                                ob[:, :, c, :],
                                ps[:].rearrange("p (t b) -> p t b", b=BL),
                                bias_sb[:, c:c + 1],
                                None,
                                op0=OP.add,
                            )
                        nc.sync.dma_start(
                            xw_t[:, n * TB:(n + 1) * TB, :, :], ob[:]
                        )

            with tc.tile_pool(name="p1x", bufs=1) as xp:
                x_sb = xp.tile([128, KIN, NT], bf)
                nc.sync.dma_start(
                    x_sb[:], xt_d.ap().rearrange("k p n -> p k n")
                )
                bulk_phase(
                    w5_d,
                    lambda k, n: x_sb[:, k, n * 512:(n + 1) * 512],
                    xw5_t, b5_sb, KIN,
                )

            # ---------- GRU recurrence (used for both layers) ----------
            def gru_phase(u_d, xw_t, g_t, brh_sb, tag):
                nc.vector.memset(h_f32[:], 0.0)
                nc.vector.memset(h_bf[:], 0.0)  # carry: h_{t-1} for uu==0
                with (
                    tc.tile_pool(name=tag + "u", bufs=1) as up,
                    tc.tile_pool(name=tag + "ps", bufs=2, space="PSUM") as psp,
                    tc.tile_pool(name=tag + "wk", bufs=3) as wk,
                    tc.tile_pool(name=tag + "xw", bufs=2) as xwp,
                    tc.tile_pool(name=tag + "hist", bufs=2) as hp,
                ):
                    u_sb = up.tile([128, KH, G3], bf, tag="usb")
                    nc.sync.dma_start(
                        u_sb[:], u_d.ap().rearrange("k p g -> p k g")
                    )
                    with tc.For_i(
                        0, T, UNROLL,
                        hint_engines=(ET.PE, ET.DVE, ET.Activation),
                    ) as iv:
                        hist = hp.tile([128, KH, UNROLL, BL], bf, tag="hist")
                        xwblk = xwp.tile([128, UNROLL, NCH, BL], bf, tag="xwt")
                        nc.sync.dma_start(
                            xwblk[:], xw_t[:, ds(iv, UNROLL), :, :]
                        )
                        for uu in range(UNROLL):
                            ps_zr = psp.tile([128, 8 * BL], f32, tag="pszr")
                            ps_ih = psp.tile([128, 4 * BL], f32, tag="psih")
                            for c in range(NCH):
                                tgt = ps_zr if c < 8 else ps_ih
                                col = (c if c < 8 else c - 8) * BL
                                for k in range(KH):
                                    nc.tensor.matmul(
                                        tgt[:, col:col + BL],
                                        u_sb[:, k, c * 128:(c + 1) * 128],
                                        h_bf[:, k, :],
                                        start=(c in (0, 8) and k == 0),
                                        stop=(c in (7, 11) and k == KH - 1),
                                        skip_group_check=True,
                                    )
                            szr = wk.tile([128, 128], f32, tag="szr")
                            nc.vector.tensor_tensor(
                                szr[:],
                                xwblk[:, uu, 0:8, :].rearrange("p c b -> p (c b)"),
                                ps_zr[:], op=OP.add,
                            )
                            zrg = wk.tile([128, 128], f32, tag="zrg")
                            nc.vector.tensor_scalar(
                                zrg[:], szr[:], 0.0, 1.0, op0=OP.max, op1=OP.min
                            )
                            if brh_sb is not None:
                                nc.vector.tensor_tensor(
                                    ps_ih[:],
                                    ps_ih[:],
                                    brh_sb[:].rearrange("p k -> p k 1")
                                    .broadcast(2, BL)
                                    .rearrange("p k b -> p (k b)"),
                                    op=OP.add,
                                )
                            q = wk.tile([128, 64], f32, tag="q")
                            nc.vector.tensor_tensor(
                                q[:], zrg[:, 64:128], ps_ih[:], op=OP.mult
                            )
                            hpre = wk.tile([128, 64], f32, tag="hpre")
                            nc.vector.tensor_tensor(
                                hpre[:], q[:],
                                xwblk[:, uu, 8:12, :].rearrange("p c b -> p (c b)"),
                                op=OP.add,
                            )
                            hh = wk.tile([128, 64], f32, tag="hh")
                            nc.scalar.activation(hh[:], hpre[:], AF.Tanh)
                            dd = wk.tile([128, 64], f32, tag="dd")
                            nc.vector.tensor_tensor(
                                dd[:], h_f32[:], hh[:], op=OP.subtract
                            )
                            ee = wk.tile([128, 64], f32, tag="ee")
                            nc.vector.tensor_tensor(
                                ee[:], zrg[:, 0:64], dd[:], op=OP.mult
                            )
                            nc.vector.tensor_tensor(
                                h_f32[:], hh[:], ee[:], op=OP.add
                            )
                            nc.vector.tensor_copy(
                                h_bf[:],
                                h_f32[:].rearrange("p (k b) -> p k b", b=BL),
                            )
                            nc.vector.tensor_copy(
                                hist[:, :, uu, :],
                                h_f32[:].rearrange("p (k b) -> p k b", b=BL),
                            )
                        nc.sync.dma_start(g_t[:, :, ds(iv, UNROLL), :], hist[:])

            gru_phase(u5_d, xw5_t, g5_t, brh5_sb if has_brh else None, "g5")

            # ---------- Phase 3: xw6 = (g5 @ W6) * scale + bias ----------
            with tc.tile_pool(name="p3x", bufs=1) as xp3:
                g5_sb = xp3.tile([128, KH, T, BL], bf)
                nc.sync.dma_start(g5_sb[:], g5_t[:])
                bulk_phase(
                    w6_d,
                    lambda k, n: g5_sb[:, k, n * TB:(n + 1) * TB, :]
                    .rearrange("p t b -> p (t b)"),
                    xw6_t, b6_sb, KH,
                )

            gru_phase(u6_d, xw6_t, g6_t, brh6_sb if has_brh else None, "g6")

            # ---------- Phase 5: dec = tanh(g6 @ Wd + bd) * dec_mask ----------
            with (
                tc.tile_pool(name="p5x", bufs=1) as xp5,
                tc.tile_pool(name="p5ps", bufs=2, space="PSUM") as psp5,
                tc.tile_pool(name="p5o", bufs=1) as op5,
            ):
                g6_sb = xp5.tile([128, KH, T, BL], bf)
                nc.sync.dma_start(g6_sb[:], g6_t[:])
                wd_sb = op5.tile([128, KH], bf, tag="wd")
                nc.sync.dma_start(wd_sb[:], wd_d.ap()[:])
                dm_sb = op5.tile([128, NJ], f32, tag="dm")
                nc.sync.dma_start(dm_sb[:], dm_d.ap()[:])
                ps_d = psp5.tile([128, NJ], f32, tag="psd")
                TJ = 128 // BL  # 8 timesteps per dense chunk
                for j in range(NJ):
                    for k in range(KH):
                        nc.tensor.matmul(
                            ps_d[:, j:j + 1],
                            g6_sb[:, k, j * TJ:(j + 1) * TJ, :]
                            .rearrange("p t b -> p (t b)"),
                            wd_sb[:, k:k + 1],
                            start=(j == 0 and k == 0),
                            stop=(j == NJ - 1 and k == KH - 1),
                            skip_group_check=True,
                        )
                dec = op5.tile([128, NJ], f32, tag="dec")
                nc.scalar.activation(
                    dec[:], ps_d[:], AF.Tanh, bias=float(bd_val), scale=1.0
                )
                nc.vector.tensor_tensor(dec[:], dec[:], dm_sb[:], op=OP.mult)
                nc.sync.dma_start(out_d.ap()[:], dec[:])

    nc.compile()
    return nc


def _prep(inputs):
    """Host-side: shard on batch, permute/pad/cast into device layouts."""
    z = np.asarray(inputs["z"], np.float32)
    x2 = np.asarray(inputs["train_input_two"], np.float32)
    masks = np.asarray(inputs["masks"], np.float32)
    dmasks = np.asarray(inputs["dec_masks"], np.float32)
    W5 = np.asarray(inputs["W5"], np.float32)
    U5 = np.asarray(inputs["U5"], np.float32)
    bi5 = np.asarray(inputs["bi5"], np.float32)
    br5 = np.asarray(inputs["br5"], np.float32)
    W6 = np.asarray(inputs["W6"], np.float32)
    U6 = np.asarray(inputs["U6"], np.float32)
    bi6 = np.asarray(inputs["bi6"], np.float32)
    br6 = np.asarray(inputs["br6"], np.float32)
    Wd = np.asarray(inputs["Wd"], np.float32)
    bd = np.asarray(inputs["bd"], np.float32)

    def scale_w(W):  # scale z,r columns by 0.2 (hard-sigmoid prescale)
        Ws = W.copy()
        Ws[:, : 2 * H] *= 0.2
        return Ws

    def pack_w(W, kdim):  # [D,G3] -> [kdim,128,G3] bf16 (zero-padded)
        D = W.shape[0]
        Wp = np.zeros((kdim * 128, G3), np.float32)
        Wp[:D] = W
        return np.ascontiguousarray(
            Wp.reshape(kdim, 128, G3).astype(bf16)
        )

    def pack_bias(bi, br):  # xw-path bias, [128, NCH] (partition, chunk)
        bt = np.empty(G3, np.float32)
        bt[: 2 * H] = 0.2 * (bi[: 2 * H] + br[: 2 * H]) + 0.5
        bt[2 * H:] = bi[2 * H:]
        return np.ascontiguousarray(bt.reshape(NCH, 128).T)

    w5p = pack_w(scale_w(W5), KIN)
    u5p = pack_w(scale_w(U5), KH)
    w6p = pack_w(scale_w(W6), KH)
    u6p = pack_w(scale_w(U6), KH)
    b5p = pack_bias(bi5, br5)
    b6p = pack_bias(bi6, br6)
    brh5 = np.ascontiguousarray(br5[2 * H:].reshape(KH, 128).T)
    brh6 = np.ascontiguousarray(br6[2 * H:].reshape(KH, 128).T)
    has_brh = bool(np.any(brh5) or np.any(brh6))
    wdp = np.ascontiguousarray(Wd[:, 0].reshape(KH, 128).T.astype(bf16))

    # masked concat input, transposed: XT [384, T*BL] per core
    rep = np.broadcast_to(z[:, None, :], (B, T, LAT))
    X = np.concatenate([rep, x2], axis=-1) * masks  # [B,T,320]

    in_maps = []
    for cidx in range(NCORES):
        sl = slice(cidx * BL, (cidx + 1) * BL)
        Xc = X[sl]                                    # [BL,T,320]
        XT = np.zeros((KIN * 128, NT), np.float32)
        XT[:DIN] = Xc.transpose(2, 1, 0).reshape(DIN, NT)  # (d,t,b)
        dmc = dmasks[sl, :, 0].T.reshape(NT)          # flat t*BL+b
        in_maps.append({
            "xt_d": np.ascontiguousarray(
                XT.reshape(KIN, 128, NT).astype(bf16)),
            "w5_d": w5p, "u5_d": u5p, "w6_d": w6p, "u6_d": u6p,
            "b5_d": b5p, "b6_d": b6p,
            "br5_d": brh5, "br6_d": brh6,
            "wd_d": wdp,
            "dm_d": np.ascontiguousarray(dmc.reshape(NJ, 128).T),
        })
    return in_maps, has_brh, float(bd.reshape(-1)[0])


def kernel(**inputs):
    from concourse.bass_utils import run_bass_kernel_spmd

    in_maps, has_brh, bd_val = _prep(inputs)
    key = (has_brh, bd_val)
    if key not in _CACHE:
        _CACHE[key] = _build(bd_val, has_brh)
    nc = _CACHE[key]
    res = run_bass_kernel_spmd(nc, in_maps, core_ids=list(range(NCORES)))
    out = np.empty((B, T, 1), np.float32)
    for cidx in range(NCORES):
        o = res.results[cidx]["out_d"]                # [128, NJ]
        flat = o.T.reshape(NT)                        # flat = t*BL + b
        out[cidx * BL:(cidx + 1) * BL, :, 0] = flat.reshape(T, BL).T
    return out


# revision 23
# speedup vs baseline: 1.0511x; 1.0511x over previous
"""Trainium2 Bass kernel for a 2-layer GRU decoder (B=128, T=512, H=512).

Sharding: data-parallel on batch across 8 cores (16 batch elems/core).
Recurrence layout: hidden state kept transposed
[128 partitions = hidden-dim within chunk, kappa(4) chunks, batch(16)];
U is the stationary matmul operand in bf16 (fast weight load), h the
moving operand, so gate math runs in the same layout the next step's
matmul consumes — no per-step transposes.
"""

import numpy as np
import ml_dtypes

B, T, LAT, F2, H = 128, 512, 256, 64, 512
DIN = LAT + F2          # 320
G3 = 3 * H              # 1536
NCORES = 8
BL = B // NCORES        # 16 batch / core
NT = T * BL             # 8192 flat (t,b) rows per core
NCH = G3 // 128         # 12 output-dim chunks
KH = H // 128           # 4 hidden-dim chunks
KIN = 3                 # padded 384 input-dim chunks
NBULK = NT // 512       # 16 bulk column chunks of 512
TB = T // NBULK         # 32 timesteps per bulk chunk
NJ = NT // 128          # 64 dense output chunks
UNROLL = 8

bf16 = ml_dtypes.bfloat16
_CACHE = {}


def _build(bd_val, has_brh):
    import concourse.bass as bass
    import concourse.tile as tile
    import concourse.mybir as mybir
    from concourse import bacc
    from concourse.bass import ds

    f32 = mybir.dt.float32
    bf = mybir.dt.bfloat16
    AF = mybir.ActivationFunctionType
    OP = mybir.AluOpType
    ET = mybir.EngineType

    nc = bacc.Bacc(None, target_bir_lowering=False, debug=False)

    xt_d = nc.dram_tensor("xt_d", [KIN, 128, NT], bf, kind="ExternalInput")
    w5_d = nc.dram_tensor("w5_d", [KIN, 128, G3], bf, kind="ExternalInput")
    u5_d = nc.dram_tensor("u5_d", [KH, 128, G3], bf, kind="ExternalInput")
    w6_d = nc.dram_tensor("w6_d", [KH, 128, G3], bf, kind="ExternalInput")
    u6_d = nc.dram_tensor("u6_d", [KH, 128, G3], bf, kind="ExternalInput")
    b5_d = nc.dram_tensor("b5_d", [128, NCH], f32, kind="ExternalInput")
    b6_d = nc.dram_tensor("b6_d", [128, NCH], f32, kind="ExternalInput")
    br5_d = nc.dram_tensor("br5_d", [128, KH], f32, kind="ExternalInput")
    br6_d = nc.dram_tensor("br6_d", [128, KH], f32, kind="ExternalInput")
    wd_d = nc.dram_tensor("wd_d", [128, KH], bf, kind="ExternalInput")
    dm_d = nc.dram_tensor("dm_d", [128, NJ], f32, kind="ExternalInput")
    out_d = nc.dram_tensor("out_d", [128, NJ], f32, kind="ExternalOutput")

    with tile.TileContext(nc) as tc:
        import contextlib
        stack = contextlib.ExitStack()
        with stack:
            drp = stack.enter_context(tc.tile_pool(name="dram", bufs=1, space="DRAM"))
            xw5_t = drp.tile([128, T, NCH, BL], bf)
            xw6_t = drp.tile([128, T, NCH, BL], bf)
            g5_t = drp.tile([128, KH, T, BL], bf)
            g6_t = drp.tile([128, KH, T, BL], bf)

            pp = stack.enter_context(tc.tile_pool(name="persist", bufs=1))
            h_f32 = pp.tile([128, KH * BL], f32, tag="hf")
            h_bf = pp.tile([128, KH, BL], bf, tag="hb")
            b5_sb = pp.tile([128, NCH], f32, tag="b5")
            b6_sb = pp.tile([128, NCH], f32, tag="b6")
            nc.sync.dma_start(b5_sb[:], b5_d.ap()[:])
            nc.sync.dma_start(b6_sb[:], b6_d.ap()[:])
            # pre-touch on DVE so later ops need no extra DMA sync waits
            scr = pp.tile([128, 1], f32, tag="scr")
            nc.vector.tensor_tensor(
                scr[:], b5_sb[:, 0:1], b6_sb[:, 0:1],
                op=mybir.AluOpType.add,
            )
            if has_brh:
                brh5_sb = pp.tile([128, KH], f32, tag="brh5")
                brh6_sb = pp.tile([128, KH], f32, tag="brh6")
                nc.sync.dma_start(brh5_sb[:], br5_d.ap()[:])
                nc.sync.dma_start(brh6_sb[:], br6_d.ap()[:])

            # ---------- Phase 1: xw5 = (X @ W5) * scale + bias ----------
            def bulk_phase(w_d, src_sb_getter, xw_t, bias_sb, kdim):
                with (
                    tc.tile_pool(name="blkw", bufs=1) as wp,
                    tc.tile_pool(name="blkps", bufs=4, space="PSUM") as psp,
                    tc.tile_pool(name="blko", bufs=2) as op_,
                ):
                    w_sb = wp.tile([128, kdim, G3], bf, tag="wsb")
                    nc.sync.dma_start(
                        w_sb[:], w_d.ap().rearrange("k p g -> p k g")
                    )
                    for n in range(NBULK):
                        ob = op_.tile([128, TB, NCH, BL], bf, tag="ob")
                        for c in range(NCH):
                            ps = psp.tile([128, 512], f32, tag="ps")
                            for k in range(kdim):
                                nc.tensor.matmul(
                                    ps[:],
                                    w_sb[:, k, c * 128:(c + 1) * 128],
                                    src_sb_getter(k, n),
                                    start=(k == 0),
                                    stop=(k == kdim - 1),
                                )
                            nc.vector.tensor_scalar(
                                ob[:, :, c, :],
                                ps[:].rearrange("p (t b) -> p t b", b=BL),
                                bias_sb[:, c:c + 1],
                                None,
                                op0=OP.add,
                            )
                        nc.sync.dma_start(
                            xw_t[:, n * TB:(n + 1) * TB, :, :], ob[:]
                        )

            with tc.tile_pool(name="p1x", bufs=1) as xp:
                x_sb = xp.tile([128, KIN, NT], bf)
                nc.sync.dma_start(
                    x_sb[:], xt_d.ap().rearrange("k p n -> p k n")
                )
                bulk_phase(
                    w5_d,
                    lambda k, n: x_sb[:, k, n * 512:(n + 1) * 512],
                    xw5_t, b5_sb, KIN,
                )

            # ---------- GRU recurrence (used for both layers) ----------
            def gru_phase(u_d, xw_t, g_t, brh_sb, tag):
                nc.vector.memset(h_f32[:], 0.0)
                nc.vector.memset(h_bf[:], 0.0)  # carry: h_{t-1} for uu==0
                with (
                    tc.tile_pool(name=tag + "u", bufs=1) as up,
                    tc.tile_pool(name=tag + "ps", bufs=2, space="PSUM") as psp,
                    tc.tile_pool(name=tag + "wk", bufs=3) as wk,
                    tc.tile_pool(name=tag + "xw", bufs=2) as xwp,
                    tc.tile_pool(name=tag + "hist", bufs=2) as hp,
                ):
                    u_sb = up.tile([128, KH, G3], bf, tag="usb")
                    nc.sync.dma_start(
                        u_sb[:], u_d.ap().rearrange("k p g -> p k g")
                    )
                    with tc.For_i(
                        0, T, UNROLL,
                        hint_engines=(ET.PE, ET.DVE, ET.Activation),
                    ) as iv:
                        hist = hp.tile([128, KH, UNROLL, BL], bf, tag="hist")
                        xwblk = xwp.tile([128, UNROLL, NCH, BL], bf, tag="xwt")
                        nc.sync.dma_start(
                            xwblk[:], xw_t[:, ds(iv, UNROLL), :, :]
                        )
                        for uu in range(UNROLL):
                            ps_zr = psp.tile([128, 8 * BL], f32, tag="pszr")
                            ps_ih = psp.tile([128, 4 * BL], f32, tag="psih")
                            for c in range(NCH):
                                tgt = ps_zr if c < 8 else ps_ih
                                col = (c if c < 8 else c - 8) * BL
                                for k in range(KH):
                                    nc.tensor.matmul(
                                        tgt[:, col:col + BL],
                                        u_sb[:, k, c * 128:(c + 1) * 128],
                                        (h_bf[:, k, :] if uu == 0
                                         else hist[:, k, uu - 1, :]),
                                        start=(c in (0, 8) and k == 0),
                                        stop=(c in (7, 11) and k == KH - 1),
                                        skip_group_check=True,
                                    )
                            szr = wk.tile([128, 128], f32, tag="szr")
                            nc.vector.tensor_tensor(
                                szr[:],
                                xwblk[:, uu, 0:8, :].rearrange("p c b -> p (c b)"),
                                ps_zr[:], op=OP.add,
                            )
                            zrg = wk.tile([128, 128], f32, tag="zrg")
                            nc.vector.tensor_scalar(
                                zrg[:], szr[:], 0.0, 1.0, op0=OP.max, op1=OP.min
                            )
                            if brh_sb is not None:
                                nc.vector.tensor_tensor(
                                    ps_ih[:],
                                    ps_ih[:],
                                    brh_sb[:].rearrange("p k -> p k 1")
                                    .broadcast(2, BL)
                                    .rearrange("p k b -> p (k b)"),
                                    op=OP.add,
                                )
                            q = wk.tile([128, 64], f32, tag="q")
                            nc.vector.tensor_tensor(
                                q[:], zrg[:, 64:128], ps_ih[:], op=OP.mult
                            )
                            hpre = wk.tile([128, 64], f32, tag="hpre")
                            nc.vector.tensor_tensor(
                                hpre[:], q[:],
                                xwblk[:, uu, 8:12, :].rearrange("p c b -> p (c b)"),
                                op=OP.add,
                            )
                            hh = wk.tile([128, 64], f32, tag="hh")
                            nc.scalar.activation(hh[:], hpre[:], AF.Tanh)
                            # runs in tanh's shadow on DVE:
                            ug = wk.tile([128, 64], f32, tag="ug")
                            nc.vector.tensor_tensor(
                                ug[:], zrg[:, 0:64], h_f32[:], op=OP.mult
                            )
                            vg = wk.tile([128, 64], f32, tag="vg")
                            nc.vector.tensor_scalar(
                                vg[:], zrg[:, 0:64], -1.0, 1.0,
                                op0=OP.mult, op1=OP.add,
                            )
                            # post-tanh chain: two ops to h', one cast
                            ee = wk.tile([128, 64], f32, tag="ee")
                            nc.vector.tensor_tensor(
                                ee[:], vg[:], hh[:], op=OP.mult
                            )
                            nc.vector.tensor_tensor(
                                h_f32[:], ug[:], ee[:], op=OP.add
                            )
                            nc.vector.tensor_copy(
                                hist[:, :, uu, :],
                                h_f32[:].rearrange("p (k b) -> p k b", b=BL),
                            )
                        # carry bf16 state across the back-edge for uu==0
                        nc.vector.tensor_copy(
                            h_bf[:], hist[:, :, UNROLL - 1, :]
                        )
                        nc.sync.dma_start(g_t[:, :, ds(iv, UNROLL), :], hist[:])

            gru_phase(u5_d, xw5_t, g5_t, brh5_sb if has_brh else None, "g5")

            # ---------- Phase 3: xw6 = (g5 @ W6) * scale + bias ----------
            with tc.tile_pool(name="p3x", bufs=1) as xp3:
                g5_sb = xp3.tile([128, KH, T, BL], bf)
                nc.sync.dma_start(g5_sb[:], g5_t[:])
                bulk_phase(
                    w6_d,
                    lambda k, n: g5_sb[:, k, n * TB:(n + 1) * TB, :]
                    .rearrange("p t b -> p (t b)"),
                    xw6_t, b6_sb, KH,
                )

            gru_phase(u6_d, xw6_t, g6_t, brh6_sb if has_brh else None, "g6")

            # ---------- Phase 5: dec = tanh(g6 @ Wd + bd) * dec_mask ----------
            with (
                tc.tile_pool(name="p5x", bufs=1) as xp5,
                tc.tile_pool(name="p5ps", bufs=2, space="PSUM") as psp5,
                tc.tile_pool(name="p5o", bufs=1) as op5,
            ):
                g6_sb = xp5.tile([128, KH, T, BL], bf)
                nc.sync.dma_start(g6_sb[:], g6_t[:])
                wd_sb = op5.tile([128, KH], bf, tag="wd")
                nc.sync.dma_start(wd_sb[:], wd_d.ap()[:])
                dm_sb = op5.tile([128, NJ], f32, tag="dm")
                nc.sync.dma_start(dm_sb[:], dm_d.ap()[:])
                ps_d = psp5.tile([128, NJ], f32, tag="psd")
                TJ = 128 // BL  # 8 timesteps per dense chunk
                for j in range(NJ):
                    for k in range(KH):
                        nc.tensor.matmul(
                            ps_d[:, j:j + 1],
                            g6_sb[:, k, j * TJ:(j + 1) * TJ, :]
                            .rearrange("p t b -> p (t b)"),
                            wd_sb[:, k:k + 1],
                            start=(j == 0 and k == 0),
                            stop=(j == NJ - 1 and k == KH - 1),
                            skip_group_check=True,
                        )
                dec = op5.tile([128, NJ], f32, tag="dec")
                nc.scalar.activation(
                    dec[:], ps_d[:], AF.Tanh, bias=float(bd_val), scale=1.0
                )
                nc.vector.tensor_tensor(dec[:], dec[:], dm_sb[:], op=OP.mult)
                nc.sync.dma_start(out_d.ap()[:], dec[:])

    nc.compile()
    return nc


def _prep(inputs):
    """Host-side: shard on batch, permute/pad/cast into device layouts."""
    z = np.asarray(inputs["z"], np.float32)
    x2 = np.asarray(inputs["train_input_two"], np.float32)
    masks = np.asarray(inputs["masks"], np.float32)
    dmasks = np.asarray(inputs["dec_masks"], np.float32)
    W5 = np.asarray(inputs["W5"], np.float32)
    U5 = np.asarray(inputs["U5"], np.float32)
    bi5 = np.asarray(inputs["bi5"], np.float32)
    br5 = np.asarray(inputs["br5"], np.float32)
    W6 = np.asarray(inputs["W6"], np.float32)
    U6 = np.asarray(inputs["U6"], np.float32)
    bi6 = np.asarray(inputs["bi6"], np.float32)
    br6 = np.asarray(inputs["br6"], np.float32)
    Wd = np.asarray(inputs["Wd"], np.float32)
    bd = np.asarray(inputs["bd"], np.float32)

    def scale_w(W):  # scale z,r columns by 0.2 (hard-sigmoid prescale)
        Ws = W.copy()
        Ws[:, : 2 * H] *= 0.2
        return Ws

    def pack_w(W, kdim):  # [D,G3] -> [kdim,128,G3] bf16 (zero-padded)
        D = W.shape[0]
        Wp = np.zeros((kdim * 128, G3), np.float32)
        Wp[:D] = W
        return np.ascontiguousarray(
            Wp.reshape(kdim, 128, G3).astype(bf16)
        )

    def pack_bias(bi, br):  # xw-path bias, [128, NCH] (partition, chunk)
        bt = np.empty(G3, np.float32)
        bt[: 2 * H] = 0.2 * (bi[: 2 * H] + br[: 2 * H]) + 0.5
        bt[2 * H:] = bi[2 * H:]
        return np.ascontiguousarray(bt.reshape(NCH, 128).T)

    w5p = pack_w(scale_w(W5), KIN)
    u5p = pack_w(scale_w(U5), KH)
    w6p = pack_w(scale_w(W6), KH)
    u6p = pack_w(scale_w(U6), KH)
    b5p = pack_bias(bi5, br5)
    b6p = pack_bias(bi6, br6)
    brh5 = np.ascontiguousarray(br5[2 * H:].reshape(KH, 128).T)
    brh6 = np.ascontiguousarray(br6[2 * H:].reshape(KH, 128).T)
    has_brh = bool(np.any(brh5) or np.any(brh6))
    wdp = np.ascontiguousarray(Wd[:, 0].reshape(KH, 128).T.astype(bf16))

    # masked concat input, transposed: XT [384, T*BL] per core
    rep = np.broadcast_to(z[:, None, :], (B, T, LAT))
    X = np.concatenate([rep, x2], axis=-1) * masks  # [B,T,320]

    in_maps = []
    for cidx in range(NCORES):
        sl = slice(cidx * BL, (cidx + 1) * BL)
        Xc = X[sl]                                    # [BL,T,320]
        XT = np.zeros((KIN * 128, NT), np.float32)
        XT[:DIN] = Xc.transpose(2, 1, 0).reshape(DIN, NT)  # (d,t,b)
        dmc = dmasks[sl, :, 0].T.reshape(NT)          # flat t*BL+b
        in_maps.append({
            "xt_d": np.ascontiguousarray(
                XT.reshape(KIN, 128, NT).astype(bf16)),
            "w5_d": w5p, "u5_d": u5p, "w6_d": w6p, "u6_d": u6p,
            "b5_d": b5p, "b6_d": b6p,
            "br5_d": brh5, "br6_d": brh6,
            "wd_d": wdp,
            "dm_d": np.ascontiguousarray(dmc.reshape(NJ, 128).T),
        })
    return in_maps, has_brh, float(bd.reshape(-1)[0])


def kernel(**inputs):
    from concourse.bass_utils import run_bass_kernel_spmd

    in_maps, has_brh, bd_val = _prep(inputs)
    key = (has_brh, bd_val)
    if key not in _CACHE:
        _CACHE[key] = _build(bd_val, has_brh)
    nc = _CACHE[key]
    res = run_bass_kernel_spmd(nc, in_maps, core_ids=list(range(NCORES)))
    out = np.empty((B, T, 1), np.float32)
    for cidx in range(NCORES):
        o = res.results[cidx]["out_d"]                # [128, NJ]
        flat = o.T.reshape(NT)                        # flat = t*BL + b
        out[cidx * BL:(cidx + 1) * BL, :, 0] = flat.reshape(T, BL).T
    return out


# revision 26
# speedup vs baseline: 1.0649x; 1.0132x over previous
"""Trainium2 Bass kernel for a 2-layer GRU decoder (B=128, T=512, H=512).

Sharding: data-parallel on batch across 8 cores (16 batch elems/core).
Recurrence layout: hidden state kept transposed
[128 partitions = hidden-dim within chunk, kappa(4) chunks, batch(16)];
U is the stationary matmul operand in bf16 (fast weight load), h the
moving operand, so gate math runs in the same layout the next step's
matmul consumes — no per-step transposes.
"""

import numpy as np
import ml_dtypes

B, T, LAT, F2, H = 128, 512, 256, 64, 512
DIN = LAT + F2          # 320
G3 = 3 * H              # 1536
NCORES = 8
BL = B // NCORES        # 16 batch / core
NT = T * BL             # 8192 flat (t,b) rows per core
NCH = G3 // 128         # 12 output-dim chunks
KH = H // 128           # 4 hidden-dim chunks
KIN = 3                 # padded 384 input-dim chunks
NBULK = NT // 512       # 16 bulk column chunks of 512
TB = T // NBULK         # 32 timesteps per bulk chunk
NJ = NT // 128          # 64 dense output chunks
UNROLL = 8

bf16 = ml_dtypes.bfloat16
_CACHE = {}


def _build(bd_val, has_brh):
    import concourse.bass as bass
    import concourse.tile as tile
    import concourse.mybir as mybir
    from concourse import bacc
    from concourse.bass import ds

    f32 = mybir.dt.float32
    bf = mybir.dt.bfloat16
    AF = mybir.ActivationFunctionType
    OP = mybir.AluOpType
    ET = mybir.EngineType

    nc = bacc.Bacc(None, target_bir_lowering=False, debug=False)

    xt_d = nc.dram_tensor("xt_d", [KIN, 128, NT], bf, kind="ExternalInput")
    w5_d = nc.dram_tensor("w5_d", [KIN, 128, G3], bf, kind="ExternalInput")
    u5_d = nc.dram_tensor("u5_d", [KH, 128, G3], bf, kind="ExternalInput")
    w6_d = nc.dram_tensor("w6_d", [KH, 128, G3], bf, kind="ExternalInput")
    u6_d = nc.dram_tensor("u6_d", [KH, 128, G3], bf, kind="ExternalInput")
    b5_d = nc.dram_tensor("b5_d", [128, NCH], f32, kind="ExternalInput")
    b6_d = nc.dram_tensor("b6_d", [128, NCH], f32, kind="ExternalInput")
    br5_d = nc.dram_tensor("br5_d", [128, KH], f32, kind="ExternalInput")
    br6_d = nc.dram_tensor("br6_d", [128, KH], f32, kind="ExternalInput")
    wd_d = nc.dram_tensor("wd_d", [128, KH], bf, kind="ExternalInput")
    dm_d = nc.dram_tensor("dm_d", [128, NJ], f32, kind="ExternalInput")
    out_d = nc.dram_tensor("out_d", [128, NJ], f32, kind="ExternalOutput")

    with tile.TileContext(nc) as tc:
        import contextlib
        stack = contextlib.ExitStack()
        with stack:
            drp = stack.enter_context(tc.tile_pool(name="dram", bufs=1, space="DRAM"))
            xw5_t = drp.tile([128, T, NCH, BL], bf)
            xw6_t = drp.tile([128, T, NCH, BL], bf)
            g5_t = drp.tile([128, KH, T, BL], bf)
            g6_t = drp.tile([128, KH, T, BL], bf)

            pp = stack.enter_context(tc.tile_pool(name="persist", bufs=1))
            h_f32 = pp.tile([128, KH * BL], f32, tag="hf")
            h_bf = pp.tile([128, KH, BL], bf, tag="hb")
            b5_sb = pp.tile([128, NCH], f32, tag="b5")
            b6_sb = pp.tile([128, NCH], f32, tag="b6")
            nc.sync.dma_start(b5_sb[:], b5_d.ap()[:])
            nc.sync.dma_start(b6_sb[:], b6_d.ap()[:])
            # pre-touch on DVE so later ops need no extra DMA sync waits
            scr = pp.tile([128, 1], f32, tag="scr")
            nc.vector.tensor_tensor(
                scr[:], b5_sb[:, 0:1], b6_sb[:, 0:1],
                op=mybir.AluOpType.add,
            )
            if has_brh:
                brh5_sb = pp.tile([128, KH], f32, tag="brh5")
                brh6_sb = pp.tile([128, KH], f32, tag="brh6")
                nc.sync.dma_start(brh5_sb[:], br5_d.ap()[:])
                nc.sync.dma_start(brh6_sb[:], br6_d.ap()[:])

            # ---------- Phase 1: xw5 = (X @ W5) * scale + bias ----------
            def bulk_phase(w_d, src_sb_getter, xw_t, bias_sb, kdim):
                with (
                    tc.tile_pool(name="blkw", bufs=1) as wp,
                    tc.tile_pool(name="blkps", bufs=4, space="PSUM") as psp,
                    tc.tile_pool(name="blko", bufs=2) as op_,
                ):
                    w_sb = wp.tile([128, kdim, G3], bf, tag="wsb")
                    nc.sync.dma_start(
                        w_sb[:], w_d.ap().rearrange("k p g -> p k g")
                    )
                    for n in range(NBULK):
                        ob = op_.tile([128, TB, NCH, BL], bf, tag="ob")
                        for c in range(NCH):
                            ps = psp.tile([128, 512], f32, tag="ps")
                            for k in range(kdim):
                                nc.tensor.matmul(
                                    ps[:],
                                    w_sb[:, k, c * 128:(c + 1) * 128],
                                    src_sb_getter(k, n),
                                    start=(k == 0),
                                    stop=(k == kdim - 1),
                                )
                            nc.vector.tensor_scalar(
                                ob[:, :, c, :],
                                ps[:].rearrange("p (t b) -> p t b", b=BL),
                                bias_sb[:, c:c + 1],
                                None,
                                op0=OP.add,
                            )
                        nc.sync.dma_start(
                            xw_t[:, n * TB:(n + 1) * TB, :, :], ob[:]
                        )

            with tc.tile_pool(name="p1x", bufs=1) as xp:
                x_sb = xp.tile([128, KIN, NT], bf)
                nc.sync.dma_start(
                    x_sb[:], xt_d.ap().rearrange("k p n -> p k n")
                )
                bulk_phase(
                    w5_d,
                    lambda k, n: x_sb[:, k, n * 512:(n + 1) * 512],
                    xw5_t, b5_sb, KIN,
                )

            # ---------- GRU recurrence (used for both layers) ----------
            def gru_phase(u_d, xw_t, g_t, brh_sb, tag):
                nc.vector.memset(h_f32[:], 0.0)
                nc.vector.memset(h_bf[:], 0.0)  # carry: h_{t-1} for uu==0
                with (
                    tc.tile_pool(name=tag + "u", bufs=1) as up,
                    tc.tile_pool(name=tag + "ps", bufs=2, space="PSUM") as psp,
                    tc.tile_pool(name=tag + "wk", bufs=3) as wk,
                    tc.tile_pool(name=tag + "xw", bufs=2) as xwp,
                    tc.tile_pool(name=tag + "hist", bufs=2) as hp,
                ):
                    u_sb = up.tile([128, KH, G3], bf, tag="usb")
                    nc.sync.dma_start(
                        u_sb[:], u_d.ap().rearrange("k p g -> p k g")
                    )
                    with tc.For_i(
                        0, T, UNROLL,
                        hint_engines=(ET.PE, ET.DVE, ET.Activation),
                    ) as iv:
                        hist = hp.tile([128, KH, UNROLL, BL], bf, tag="hist")
                        xwblk = xwp.tile([128, UNROLL, NCH, BL], bf, tag="xwt")
                        nc.sync.dma_start(
                            xwblk[:], xw_t[:, ds(iv, UNROLL), :, :]
                        )
                        for uu in range(UNROLL):
                            ps_z = psp.tile([128, 4 * BL], f32, tag="psz")
                            ps_r = psp.tile([128, 4 * BL], f32, tag="psr")
                            ps_ih = psp.tile([128, 4 * BL], f32, tag="psih")

                            def mm_gate(tgt, c0):
                                for cc in range(4):
                                    c = c0 + cc
                                    col = cc * BL
                                    for k in range(KH):
                                        nc.tensor.matmul(
                                            tgt[:, col:col + BL],
                                            u_sb[:, k, c * 128:(c + 1) * 128],
                                            (h_bf[:, k, :] if uu == 0
                                             else hist[:, k, uu - 1, :]),
                                            start=(cc == 0 and k == 0),
                                            stop=(cc == 3 and k == KH - 1),
                                            skip_group_check=True,
                                        )

                            # r first: its gate math overlaps the z/ih matmuls
                            mm_gate(ps_r, 4)
                            sr = wk.tile([128, 64], f32, tag="sr")
                            nc.vector.tensor_tensor(
                                sr[:],
                                xwblk[:, uu, 4:8, :].rearrange("p c b -> p (c b)"),
                                ps_r[:], op=OP.add,
                            )
                            rg = wk.tile([128, 64], f32, tag="rg")
                            nc.vector.tensor_scalar(
                                rg[:], sr[:], 0.0, 1.0, op0=OP.max, op1=OP.min
                            )
                            mm_gate(ps_z, 0)
                            sz = wk.tile([128, 64], f32, tag="sz")
                            nc.vector.tensor_tensor(
                                sz[:],
                                xwblk[:, uu, 0:4, :].rearrange("p c b -> p (c b)"),
                                ps_z[:], op=OP.add,
                            )
                            zg = wk.tile([128, 64], f32, tag="zg")
                            nc.vector.tensor_scalar(
                                zg[:], sz[:], 0.0, 1.0, op0=OP.max, op1=OP.min
                            )
                            mm_gate(ps_ih, 8)
                            if brh_sb is not None:
                                nc.vector.tensor_tensor(
                                    ps_ih[:],
                                    ps_ih[:],
                                    brh_sb[:].rearrange("p k -> p k 1")
                                    .broadcast(2, BL)
                                    .rearrange("p k b -> p (k b)"),
                                    op=OP.add,
                                )
                            q = wk.tile([128, 64], f32, tag="q")
                            nc.vector.tensor_tensor(
                                q[:], rg[:], ps_ih[:], op=OP.mult
                            )
                            hpre = wk.tile([128, 64], f32, tag="hpre")
                            nc.vector.tensor_tensor(
                                hpre[:], q[:],
                                xwblk[:, uu, 8:12, :].rearrange("p c b -> p (c b)"),
                                op=OP.add,
                            )
                            hh = wk.tile([128, 64], f32, tag="hh")
                            nc.scalar.activation(hh[:], hpre[:], AF.Tanh)
                            # runs in tanh's shadow on DVE:
                            ug = wk.tile([128, 64], f32, tag="ug")
                            nc.vector.tensor_tensor(
                                ug[:], zg[:], h_f32[:], op=OP.mult
                            )
                            vg = wk.tile([128, 64], f32, tag="vg")
                            nc.vector.tensor_scalar(
                                vg[:], zg[:], -1.0, 1.0,
                                op0=OP.mult, op1=OP.add,
                            )
                            # post-tanh chain: two ops to h', one cast
                            ee = wk.tile([128, 64], f32, tag="ee")
                            nc.vector.tensor_tensor(
                                ee[:], vg[:], hh[:], op=OP.mult
                            )
                            nc.vector.tensor_tensor(
                                h_f32[:], ug[:], ee[:], op=OP.add
                            )
                            nc.vector.tensor_copy(
                                hist[:, :, uu, :],
                                h_f32[:].rearrange("p (k b) -> p k b", b=BL),
                            )
                        # carry bf16 state across the back-edge for uu==0
                        nc.vector.tensor_copy(
                            h_bf[:], hist[:, :, UNROLL - 1, :]
                        )
                        nc.sync.dma_start(g_t[:, :, ds(iv, UNROLL), :], hist[:])

            gru_phase(u5_d, xw5_t, g5_t, brh5_sb if has_brh else None, "g5")

            # ---------- Phase 3: xw6 = (g5 @ W6) * scale + bias ----------
            with tc.tile_pool(name="p3x", bufs=1) as xp3:
                g5_sb = xp3.tile([128, KH, T, BL], bf)
                nc.sync.dma_start(g5_sb[:], g5_t[:])
                bulk_phase(
                    w6_d,
                    lambda k, n: g5_sb[:, k, n * TB:(n + 1) * TB, :]
                    .rearrange("p t b -> p (t b)"),
                    xw6_t, b6_sb, KH,
                )

            gru_phase(u6_d, xw6_t, g6_t, brh6_sb if has_brh else None, "g6")

            # ---------- Phase 5: dec = tanh(g6 @ Wd + bd) * dec_mask ----------
            with (
                tc.tile_pool(name="p5x", bufs=1) as xp5,
                tc.tile_pool(name="p5ps", bufs=2, space="PSUM") as psp5,
                tc.tile_pool(name="p5o", bufs=1) as op5,
            ):
                g6_sb = xp5.tile([128, KH, T, BL], bf)
                nc.sync.dma_start(g6_sb[:], g6_t[:])
                wd_sb = op5.tile([128, KH], bf, tag="wd")
                nc.sync.dma_start(wd_sb[:], wd_d.ap()[:])
                dm_sb = op5.tile([128, NJ], f32, tag="dm")
                nc.sync.dma_start(dm_sb[:], dm_d.ap()[:])
                ps_d = psp5.tile([128, NJ], f32, tag="psd")
                TJ = 128 // BL  # 8 timesteps per dense chunk
                for j in range(NJ):
                    for k in range(KH):
                        nc.tensor.matmul(
                            ps_d[:, j:j + 1],
                            g6_sb[:, k, j * TJ:(j + 1) * TJ, :]
                            .rearrange("p t b -> p (t b)"),
                            wd_sb[:, k:k + 1],
                            start=(j == 0 and k == 0),
                            stop=(j == NJ - 1 and k == KH - 1),
                            skip_group_check=True,
                        )
                dec = op5.tile([128, NJ], f32, tag="dec")
                nc.scalar.activation(
                    dec[:], ps_d[:], AF.Tanh, bias=float(bd_val), scale=1.0
                )
                nc.vector.tensor_tensor(dec[:], dec[:], dm_sb[:], op=OP.mult)
                nc.sync.dma_start(out_d.ap()[:], dec[:])

    nc.compile()
    return nc


def _prep(inputs):
    """Host-side: shard on batch, permute/pad/cast into device layouts."""
    z = np.asarray(inputs["z"], np.float32)
    x2 = np.asarray(inputs["train_input_two"], np.float32)
    masks = np.asarray(inputs["masks"], np.float32)
    dmasks = np.asarray(inputs["dec_masks"], np.float32)
    W5 = np.asarray(inputs["W5"], np.float32)
    U5 = np.asarray(inputs["U5"], np.float32)
    bi5 = np.asarray(inputs["bi5"], np.float32)
    br5 = np.asarray(inputs["br5"], np.float32)
    W6 = np.asarray(inputs["W6"], np.float32)
    U6 = np.asarray(inputs["U6"], np.float32)
    bi6 = np.asarray(inputs["bi6"], np.float32)
    br6 = np.asarray(inputs["br6"], np.float32)
    Wd = np.asarray(inputs["Wd"], np.float32)
    bd = np.asarray(inputs["bd"], np.float32)

    def scale_w(W):  # scale z,r columns by 0.2 (hard-sigmoid prescale)
        Ws = W.copy()
        Ws[:, : 2 * H] *= 0.2
        return Ws

    def pack_w(W, kdim):  # [D,G3] -> [kdim,128,G3] bf16 (zero-padded)
        D = W.shape[0]
        Wp = np.zeros((kdim * 128, G3), np.float32)
        Wp[:D] = W
        return np.ascontiguousarray(
            Wp.reshape(kdim, 128, G3).astype(bf16)
        )

    def pack_bias(bi, br):  # xw-path bias, [128, NCH] (partition, chunk)
        bt = np.empty(G3, np.float32)
        bt[: 2 * H] = 0.2 * (bi[: 2 * H] + br[: 2 * H]) + 0.5
        bt[2 * H:] = bi[2 * H:]
        return np.ascontiguousarray(bt.reshape(NCH, 128).T)

    w5p = pack_w(scale_w(W5), KIN)
    u5p = pack_w(scale_w(U5), KH)
    w6p = pack_w(scale_w(W6), KH)
    u6p = pack_w(scale_w(U6), KH)
    b5p = pack_bias(bi5, br5)
    b6p = pack_bias(bi6, br6)
    brh5 = np.ascontiguousarray(br5[2 * H:].reshape(KH, 128).T)
    brh6 = np.ascontiguousarray(br6[2 * H:].reshape(KH, 128).T)
    has_brh = bool(np.any(brh5) or np.any(brh6))
    wdp = np.ascontiguousarray(Wd[:, 0].reshape(KH, 128).T.astype(bf16))

    # masked concat input, transposed: XT [384, T*BL] per core
    rep = np.broadcast_to(z[:, None, :], (B, T, LAT))
    X = np.concatenate([rep, x2], axis=-1) * masks  # [B,T,320]

    in_maps = []
    for cidx in range(NCORES):
        sl = slice(cidx * BL, (cidx + 1) * BL)
        Xc = X[sl]                                    # [BL,T,320]
        XT = np.zeros((KIN * 128, NT), np.float32)
        XT[:DIN] = Xc.transpose(2, 1, 0).reshape(DIN, NT)  # (d,t,b)
        dmc = dmasks[sl, :, 0].T.reshape(NT)          # flat t*BL+b
        in_maps.append({
            "xt_d": np.ascontiguousarray(
                XT.reshape(KIN, 128, NT).astype(bf16)),
            "w5_d": w5p, "u5_d": u5p, "w6_d": w6p, "u6_d": u6p,
            "b5_d": b5p, "b6_d": b6p,
            "br5_d": brh5, "br6_d": brh6,
            "wd_d": wdp,
            "dm_d": np.ascontiguousarray(dmc.reshape(NJ, 128).T),
        })
    return in_maps, has_brh, float(bd.reshape(-1)[0])


def kernel(**inputs):
    from concourse.bass_utils import run_bass_kernel_spmd

    in_maps, has_brh, bd_val = _prep(inputs)
    key = (has_brh, bd_val)
    if key not in _CACHE:
        _CACHE[key] = _build(bd_val, has_brh)
    nc = _CACHE[key]
    res = run_bass_kernel_spmd(nc, in_maps, core_ids=list(range(NCORES)))
    out = np.empty((B, T, 1), np.float32)
    for cidx in range(NCORES):
        o = res.results[cidx]["out_d"]                # [128, NJ]
        flat = o.T.reshape(NT)                        # flat = t*BL + b
        out[cidx * BL:(cidx + 1) * BL, :, 0] = flat.reshape(T, BL).T
    return out
